# revision 24
# baseline (speedup 1.0000x reference)
"""Trainium2 Bass kernel: VAE-style AttnBlock.

  y = x + proj( attention( q(gn(x)), k(gn(x)), v(gn(x)) ) )

  x: [2, 512, 64, 64] f32, gn = GroupNorm(8 groups, eps=1e-6),
  q/k/v/proj = 1x1 convs (512x512), attention over the 4096 spatial
  positions with softmax along the key axis, scale = 512**-0.5.

Sharding: 8 cores = (batch b, query-block qb); each core computes the
softmax rows for its 1024 query positions of batch b against the full
K/V of that batch.  Conv weights replicated.  The proj-fused V tensor
VT is SHARDED: each core computes VT only for its own 1024 positions
(from xqc, the per-core query-block input, so the SPMD program needs
no per-core offsets) and a 4-core HBM AllGather assembles the full
[4096, 512] VT during the S phase, saving ~10us of PE time per core
vs recomputing all of it.

Folding (host side, exact f32/f64): GroupNorm stats (mean/var per
group per batch) fold into the conv weights; Wq^T@Wk pre-multiplies
into one bilinear matrix A so the S matmul needs a conv on the query
side only; Wp@Wv pre-multiplies so AV directly yields the projected
output.  The k-side bias and v-bias fold into per-query constants /
the output bias (softmax over keys is invariant to per-query shifts).

Device work is exactly the O(n C^2) convs and O(n^2 C) attention:
  VTl = xqc^T wv8          (proj-fused V, own block; AllGather -> VT)
  q8 = s/16 (wa8^T xqc + bqe)                (fused Q, fp8)
  S^T = x8^T q8 ; at = exp(S/sqrt(C) - 3)    (fp8)
  cs  = ones^T at          (softmax normalizer, PE-accumulated)
  O   = (VT^T at) / cs + bpe + x             (f32 epilogue)

All large matmuls run fp8 (e4m3) with DoubleRow perf mode - the PE
packs two fp8 weights per cell, contracting 256 rows per pass at ~2x
the bf16 rate (~1 column/cycle at 2.4 GHz).  Operands use the
DoubleRow 3D AP [K=128, 2, free] with 16B-aligned pair steps; channels
pair (c, c+128) inside chunk pairs so each pair lives in one
partition.  The folded weights ship pre-scaled by 16 (entries
~N(0, 0.002) would otherwise quantize into fp8 subnormals); 1/16 is
folded into the f32 epilogues.  The -3 shift keeps exp below 48 (e4m3
max 240; logits are ~N(0,1), max ~6.8) and cancels exactly in the
softmax ratio.  exp runs 1024 wide from a two-bank PSUM tile to
amortize the ACT instruction overhead.  Accumulation is fp32 PSUM
everywhere; the residual input and the epilogue output ship bf16
(host upcasts) to halve those DMAs.

DMA: everything rides the two HARDWARE-dynamic rings (sync/SP and
scalar/Act); the gpsimd/Pool queue is software-dynamic (slow) and
only triggers the collective.  Transfers pending on one ring
round-robin its bandwidth and the early wire runs at only ~150-200
GB/s, so the startup-critical bytes are minimized (wa8 ships m-major:
the first q8 matmul needs only 64K of weights + the 512K xqc) and
later bulk is released ring-program-ordered behind tiny SBUF->SBUF
gate DMAs that wait on mid-kernel tile writes.

S and AV both run ih=1 before ih=0: AV[1] interleaves into S[0]'s
ACT-wait bubbles, and the final AV[0] epilogue chain pipelines behind
its matmuls because rb[0] is ready mid-stream.  The last chunk's
epilogue + store run in two 256-column halves on both rings.

End-to-end rel l2 vs the f32 reference: ~4.2e-3 (gate 2e-2).
"""

import numpy as np
import ml_dtypes

import concourse.bacc as bacc
import concourse.tile as tile
from concourse import mybir
from concourse import bass_utils

B, C, H, W = 2, 512, 64, 64
HW = H * W              # 4096 spatial positions
P = 128                 # partitions
KC = C // P             # 4 channel chunks; chunk k = (cp, i) = (k//2, k%2)
NCP = 2                 # chunk pairs (DoubleRow contraction = 256 channels)
NCORES = 8
NB = NCORES // B        # 4 query blocks / cores per batch
QB = HW // NB           # 1024 query positions per core
NIH = 2                 # query halves of 512
G = 8                   # groups
GSZ = C // G            # 64 channels / group
EPS = 1e-6
SCALE = float(C) ** -0.5
NJT = HW // P           # 32 key tiles of 128
NTP = NJT // 2          # 16 key tile-pairs (DoubleRow)
NTPL = QB // 256        # 4 local (own-block) tile-pairs
NCH = 8                 # x8 DMA position-chunks
CHW = HW // NCH         # 512 positions per chunk
SHIFT = 3.0             # exp(logit - SHIFT); cancels in the softmax ratio
WS = 16.0               # host prescale of the folded weights before fp8

F32 = mybir.dt.float32
BF16 = mybir.dt.bfloat16
FP8 = mybir.dt.float8e4
NP8 = ml_dtypes.float8_e4m3
AX = mybir.AxisListType
OP = mybir.AluOpType
AF = mybir.ActivationFunctionType
DR = mybir.MatmulPerfMode.DoubleRow


def _build():
    nc = bacc.Bacc("TRN2", target_bir_lowering=False, debug=False,
                   num_devices=NCORES)

    x8_d = nc.dram_tensor("x8", [P, NCH, NCP, 2, CHW], FP8,
                          kind="ExternalInput").ap()
    xqc_d = nc.dram_tensor("xqc", [P, NCP, 2, QB], FP8,
                           kind="ExternalInput").ap()
    xq_d = nc.dram_tensor("xq", [P, KC, QB], BF16, kind="ExternalInput").ap()
    wa_d = nc.dram_tensor("wa8", [P, KC, NCP, 2, P], FP8,
                          kind="ExternalInput").ap()
    wv_d = nc.dram_tensor("wv8", [P, NCP, 2, C], FP8, kind="ExternalInput").ap()
    ones_d = nc.dram_tensor("ones8", [P, 2, P], FP8, kind="ExternalInput").ap()
    vec_d = nc.dram_tensor("vecs", [P, 4 * KC], F32, kind="ExternalInput").ap()
    vtl_d = nc.dram_tensor("vtl", [P, NTPL, 2, C], FP8, kind="Internal").ap()
    vtg_d = nc.dram_tensor("vtg", [NB, P, NTPL, 2, C], FP8,
                           kind="Internal").ap()
    out_d = nc.dram_tensor("out", [P, KC, QB], BF16, kind="ExternalOutput").ap()

    with tile.TileContext(nc) as tc:
        _body(nc, tc, x8_d, xqc_d, xq_d, wa_d, wv_d, ones_d, vec_d,
              vtl_d, vtg_d, out_d)

    nc.compile()
    return nc


def _body(nc, tc, x8_d, xqc_d, xq_d, wa_d, wv_d, ones_d, vec_d,
          vtl_d, vtg_d, out_d):
    with (
        tc.tile_pool(name="xbuf", bufs=1) as px,
        tc.tile_pool(name="vt", bufs=1) as pvt,
        tc.tile_pool(name="atb", bufs=1) as pat,
        tc.tile_pool(name="qbuf", bufs=1) as pq,
        tc.tile_pool(name="w8", bufs=2) as pw8,
        tc.tile_pool(name="xq", bufs=1) as pxq,
        tc.tile_pool(name="small", bufs=4) as ps,
    ):
        # ---- persistent tiles ------------------------------------------
        x8t = px.tile([P, NCH, NCP, 2, CHW], FP8, name="x8t")
        xqc = px.tile([P, NCP, 2, QB], FP8, tag="xqc", name="xqc")
        vt8 = pvt.tile([P, NTP, 2, C], FP8, name="vt8")
        vtl8 = pvt.tile([P, NTPL, 2, C], FP8, tag="vtl", name="vtl8")
        at8 = pat.tile([P, NIH, NTP, 2, 512], FP8, name="at8")
        q8t = pq.tile([P, NCP, 2, QB], FP8, name="q8t")
        wa8 = pw8.tile([P, KC, NCP, 2, P], FP8, tag="wa", name="wa8")
        wv8 = pw8.tile([P, NCP, 2, C], FP8, tag="wv", name="wv8")
        xq_b = pxq.tile([P, KC, QB], BF16, name="xqb")
        rb_t = [ps.tile([P, 512], F32, tag=f"rb{ih}", name=f"rb{ih}")
                for ih in range(NIH)]
        ones_t = ps.tile([P, 2, P], FP8, tag="ones", name="onest")
        vec_b = ps.tile([P, 4 * KC], F32, tag="vec", name="vecb")

        # memsets on the DVE queue (free until the q8 epilogues) so the
        # warm-up matmuls aren't serialized behind framework gpsimd work
        nsh_t = ps.tile([P, 1], F32, tag="nsh", name="nsh")
        nc.vector.memset(nsh_t[:], -float(SHIFT))
        wsrc = ps.tile([P, 2, 256], FP8, tag="wsrc", name="wsrc")
        nc.vector.memset(wsrc[:], 1.0)
        warm = ps.tile([G, 1], F32, tag="warm", name="warm")

        # startup-critical DMAs: wa8 m=0 (64K) + xqc (512K) gate the
        # first q8 matmul (~10.5us); wv8 rides the lower-priority scalar
        # ring.  x8 chunks / xq are released later behind gate DMAs.
        nc.sync.dma_start(out=wa8[:, 0], in_=wa_d[:, 0])
        nc.sync.dma_start(out=xqc[:], in_=xqc_d[:])
        for m in range(1, KC):
            nc.sync.dma_start(out=wa8[:, m], in_=wa_d[:, m])
        nc.scalar.dma_start(out=vec_b[:], in_=vec_d[:])
        nc.scalar.dma_start(out=wv8[:], in_=wv_d[:])
        # warm the exp table during the DMAs
        nc.scalar.activation(out=warm[:], in_=nsh_t[0:G, :], func=AF.Exp,
                             scale=SCALE)

        bqe_t = [vec_b[:, m:m + 1] for m in range(KC)]           # 16*(A^T t + Wk^T bq)
        sc16_t = [vec_b[:, 4 + m:5 + m] for m in range(KC)]      # s_cout / 16
        bpe_t = [vec_b[:, 8 + m:9 + m] for m in range(KC)]       # Pv t + Wp bv + bp
        bqs_t = [vec_b[:, 12 + m:13 + m] for m in range(KC)]     # bqe * sc16

        with tc.tile_pool(name="convps", bufs=4, space="PSUM") as pcv:
            # ~2.6us of throwaway matmuls on a memset tile, starting
            # right after program init while the DMAs are still in
            # flight: sustained PE activity ramps the clock to the full
            # rate before the real matmul stream begins
            for r in range(12):
                trash = pcv.tile([P, 256], F32, tag="cv", name=f"trash{r}")
                nc.tensor.matmul(trash[:], lhsT=wsrc[:, :, 0:P],
                                 rhs=wsrc[:], start=True, stop=True,
                                 perf_mode=DR)

            # ---- q8 = s/16 * (16 A_s^T xqc + 16 A^T t) -----------------
            # (first: its DVE epilogues must lead the vtl casts in the
            # DVE queue so the S matmuls aren't gated on cast backlog)
            for th in range(NIH):
                for m in range(KC):
                    qp = pcv.tile([P, 512], F32, tag="cv", name=f"qp{m}{th}")
                    for cp in range(NCP):
                        nc.tensor.matmul(
                            qp[:],
                            lhsT=wa8[:, m, cp],
                            rhs=xqc[:, cp, :, 512 * th:512 * (th + 1)],
                            start=(cp == 0), stop=(cp == NCP - 1),
                            perf_mode=DR)
                    nc.vector.tensor_scalar(
                        out=q8t[:, m // 2, m % 2, 512 * th:512 * (th + 1)],
                        in0=qp[:], scalar1=bqe_t[m], scalar2=sc16_t[m],
                        op0=OP.add, op1=OP.mult)
            # x8 ch0-3 issue once the first q8 epilogue lands (~11us):
            # an SBUF->SBUF gate DMA on sync picks up that dependency
            # and blocks the ring program until then
            gate1 = ps.tile([1, 4], FP8, tag="gate1", name="gate1")
            nc.sync.dma_start(out=gate1[:], in_=q8t[0:1, 0, 0, 0:4])
            for ch in range(4):
                nc.sync.dma_start(out=x8t[:, ch], in_=x8_d[:, ch])

            # ---- VTl = xqc^T (16 Pv_s)^T, cast to fp8 with /16 ---------
            # (own 1024 positions only; the 4-core AllGather below
            # assembles the full VT in HBM during the S phase)
            for jl in range(QB // P):
                vp = pcv.tile([P, 512], F32, tag="qcv", name=f"vp{jl}")
                for cp in range(NCP):
                    nc.tensor.matmul(
                        vp[:],
                        lhsT=xqc[:, cp, :, P * jl:P * (jl + 1)],
                        rhs=wv8[:, cp],
                        start=(cp == 0), stop=(cp == NCP - 1), perf_mode=DR)
                dst = vtl8[:, jl // 2, jl % 2, :]
                if jl % 2 == 0:
                    nc.vector.tensor_scalar_mul(dst, vp[:], 1.0 / WS)
                else:
                    nc.scalar.activation(out=dst, in_=vp[:], func=AF.Copy,
                                         scale=1.0 / WS)
                if jl == 1:
                    nc.scalar.dma_start(out=ones_t[:], in_=ones_d[:])

            # x8 ch4-7 after the last q8 epilogue (~14us)
            gate2 = ps.tile([1, 4], FP8, tag="gate2", name="gate2")
            nc.sync.dma_start(out=gate2[:], in_=q8t[0:1, 1, 1, QB - 4:QB])
            for ch in range(4, NCH):
                nc.sync.dma_start(out=x8t[:, ch], in_=x8_d[:, ch])

            # ship VTl out on scalar (free after wv8) and all-gather.
            # the gpsimd queue only triggers the collective.
            nc.scalar.dma_start(out=vtl_d, in_=vtl8[:])
            nc.gpsimd.collective_compute(
                kind="AllGather", op=OP.bypass,
                replica_groups=[[0, 1, 2, 3], [4, 5, 6, 7]],
                ins=[vtl_d], outs=[vtg_d])

            # xq (1 MiB, first needed by the AV epilogues ~55us), then
            # the gathered VT, both behind the ch4-7 issues on sync;
            # vt8's tile-pair index tp = 4*qb + t is the natural key
            # order because replica qb contributes slot qb
            nc.sync.dma_start(out=xq_b[:], in_=xq_d[:])
            for g in range(NB):
                nc.sync.dma_start(out=vt8[:, NTPL * g:NTPL * (g + 1)],
                                  in_=vtg_d[g])

        # ---- attention -------------------------------------------------
        with (
            tc.tile_pool(name="ob", bufs=8) as pob,
            tc.tile_pool(name="sps", bufs=2, space="PSUM") as psps,
            tc.tile_pool(name="csps", bufs=1, space="PSUM") as pcs,
            tc.tile_pool(name="ops", bufs=3, space="PSUM") as pops,
        ):
            # S^T (1024-wide two-bank psum) + one wide exp per tile-pair
            # + PE-accumulated row sums.  ih=1 first (see module doc).
            for ih in (1, 0):
                    i_sl = slice(512 * ih, 512 * (ih + 1))
                    cs_ps = pcs.tile([P, 512], F32, tag="cs", name=f"cs{ih}")

                    def cs_mm(tp):
                        # row-sum matmul, one tile-pair behind the exps so
                        # the PE never waits on ACT
                        nc.tensor.matmul(
                            cs_ps[:], lhsT=ones_t[:], rhs=at8[:, ih, tp],
                            start=(tp == 0), stop=(tp == NTP - 1),
                            perf_mode=DR)

                    for tp in range(NTP):
                        sp = psps.tile([P, 2, 512], F32, tag="sp",
                                       name=f"sp{ih}{tp}")
                        for i2 in range(2):
                            jt = 2 * tp + i2
                            ch, l = divmod(jt, NJT // NCH)
                            for cp in range(NCP):
                                nc.tensor.matmul(
                                    sp[:, i2, :],
                                    lhsT=x8t[:, ch, cp, :, P * l:P * (l + 1)],
                                    rhs=q8t[:, cp, :, i_sl],
                                    start=(cp == 0), stop=(cp == NCP - 1),
                                    perf_mode=DR)
                        nc.scalar.activation(
                            out=at8[:, ih, tp], in_=sp[:],
                            func=AF.Exp, scale=SCALE, bias=nsh_t[:])
                        if tp > 0:
                            cs_mm(tp - 1)
                    cs_mm(NTP - 1)
                    nc.vector.reciprocal_approx_fast(
                        out=rb_t[ih][:], in_=cs_ps[:])

            # AV (m-major; per-m epilogue + output DMA)
            for ihi, ih in enumerate((1, 0)):
                    i_sl = slice(512 * ih, 512 * (ih + 1))
                    for m in range(KC):
                        last = (ihi == NIH - 1) and (m == KC - 1)
                        o_ps = pops.tile([P, 512], F32, tag="ops",
                                         name=f"ops{ih}{m}")
                        for tp in range(NTP):
                            nc.tensor.matmul(
                                o_ps[:],
                                lhsT=vt8[:, tp, :, P * m:P * (m + 1)],
                                rhs=at8[:, ih, tp],
                                start=(tp == 0), stop=(tp == NTP - 1),
                                perf_mode=DR)
                        ob = pob.tile([P, 512], BF16, tag="ob",
                                      name=f"ob{ih}{m}")
                        halves = (2 if last else 1)
                        for hf in range(halves):
                            h_sl = slice(512 * hf // halves,
                                         512 * (hf + 1) // halves)
                            ho_sl = slice(512 * ih + h_sl.start,
                                          512 * ih + h_sl.stop)
                            nc.vector.tensor_tensor(
                                out=ob[:, h_sl], in0=o_ps[:, h_sl],
                                in1=rb_t[ih][:, h_sl], op=OP.mult)
                            nc.vector.scalar_tensor_tensor(
                                out=ob[:, h_sl], in0=ob[:, h_sl],
                                scalar=bpe_t[m], in1=xq_b[:, m, ho_sl],
                                op0=OP.add, op1=OP.add)
                            eng = nc.sync if (m + hf) % 2 == 0 else nc.scalar
                            eng.dma_start(out=out_d[:, m, ho_sl],
                                          in_=ob[:, h_sl])


_NC_CACHE = {}


def _get_nc():
    if "nc" not in _NC_CACHE:
        _NC_CACHE["nc"] = _build()
    return _NC_CACHE["nc"]


def prepare(inputs):
    x = np.ascontiguousarray(np.asarray(inputs["x"], np.float32))
    norm_w = np.asarray(inputs["norm_w"], np.float64)
    norm_b = np.asarray(inputs["norm_b"], np.float64)
    bs = {w: np.asarray(inputs["b" + w], np.float64) for w in "qkvp"}
    amat = (np.asarray(inputs["wq"], np.float64).T
            @ np.asarray(inputs["wk"], np.float64))
    pvt = (np.asarray(inputs["wp"], np.float64)
           @ np.asarray(inputs["wv"], np.float64)).T
    bqx = np.asarray(inputs["wk"], np.float64).T @ bs["q"]
    bpx = np.asarray(inputs["wp"], np.float64) @ bs["v"] + bs["p"]

    ones8 = np.ones((P, 2, P), NP8)
    # per-batch GroupNorm stats -> folded scaled weights + bias vectors
    per_b = []
    for b in range(B):
        xb = x[b].reshape(C, HW)
        xg = xb.reshape(G, -1).astype(np.float64)
        mean = xg.mean(1)
        var = xg.var(1)
        s = (norm_w / np.sqrt(var + EPS).repeat(GSZ))        # [C]
        t = norm_b - mean.repeat(GSZ) * s                    # [C]
        # m-major pair layout [p, m, cp, i, 128]: cin = cp*256 + i*128
        # + p, cout = m*128 + col (m blocks DMA separately so the first
        # q8 matmul starts after only 64K of weights)
        wa8 = np.ascontiguousarray(
            (WS * amat * s[:, None]).astype(np.float32).astype(NP8)
            .reshape(NCP, 2, P, KC, P).transpose(2, 3, 0, 1, 4))
        wv8 = np.ascontiguousarray(
            (WS * pvt * s[:, None]).astype(np.float32).astype(NP8)
            .reshape(NCP, 2, P, C).transpose(2, 0, 1, 3))
        bqe = WS * (amat.T @ t + bqx)                        # [C]
        bpe = pvt.T @ t + bpx                                # [C]
        # [P, 16]: cols 0-3 = bqe chunks, 4-7 = s/16, 8-11 = bpe,
        # 12-15 = bqe*s/16 (ACT-affine form of the q epilogue)
        vecs = np.ascontiguousarray(np.concatenate(
            [bqe.reshape(KC, P).T, (s / WS).reshape(KC, P).T,
             bpe.reshape(KC, P).T,
             (bqe * s / WS).reshape(KC, P).T], axis=1).astype(np.float32))
        # natural key order; identical for the 4 cores of the batch
        x8 = np.ascontiguousarray(
            xb.astype(NP8).reshape(NCP, 2, P, NCH, CHW)
            .transpose(2, 3, 0, 1, 4))
        per_b.append((wa8, wv8, vecs, x8, xb))

    in_maps = []
    for core in range(NCORES):
        b, qb = divmod(core, NB)
        wa8, wv8, vecs, x8, xb = per_b[b]
        blk = xb[:, qb * QB:(qb + 1) * QB]
        # the core's own block: residual (bf16) + conv input (fp8 pairs)
        xq = np.ascontiguousarray(
            blk.astype(ml_dtypes.bfloat16)
            .reshape(KC, P, QB).transpose(1, 0, 2))
        xqc = np.ascontiguousarray(
            blk.astype(NP8).reshape(NCP, 2, P, QB).transpose(2, 0, 1, 3))
        in_maps.append({
            "x8": x8, "xqc": xqc, "xq": xq, "wa8": wa8, "wv8": wv8,
            "ones8": ones8, "vecs": vecs,
        })
    return in_maps


def assemble(results):
    out = np.empty((B, C, HW), np.float32)
    for core in range(NCORES):
        b, qb = divmod(core, NB)
        # [p, k, n] -> [k*128 + p = c, n]
        arr = np.asarray(results[core]["out"], np.float32)
        out[b][:, qb * QB:(qb + 1) * QB] = (
            arr.transpose(1, 0, 2).reshape(C, QB))
    return out.reshape(B, C, H, W)


def run(inputs, **spmd_kwargs):
    in_maps = prepare(inputs)
    nc = _get_nc()
    res = bass_utils.run_bass_kernel_spmd(nc, in_maps, list(range(NCORES)),
                                          **spmd_kwargs)
    return assemble(res.results), res


def kernel(**inputs):
    out, _ = run(inputs)
    return out
